# revision 3
# baseline (speedup 1.0000x reference)
"""Trainium2 Bass kernel for batched single-head attention with projections.

Reference computation (per batch b):
    Q = q @ Wq + bq ; K = k @ Wk + bk ; V = v @ Wv + bv        (512 -> 64)
    out = softmax(Q K^T / 8) V                                  (S = 4096)

Sharding: 8 cores = 4 batches x 2 query-sequence halves. Each core gets
its q half (transposed, bf16) plus the full k,v for its batch (transposed,
bf16, duplicated across the pair of cores that share the batch).

Device-side layout trick: everything is computed in "transposed space".
  Q.T [64, 2048]  = Wq.T @ qT   (+bq per-partition)
  K.T [64, 4096]  = Wk.T @ kT   (bk dropped: softmax-invariant)
  V'  [4096, 65]  = (vT.T @ Wv_aug) + bias ; col 64 == 1.0 (denominator col)
  scores.T tile   = K.T-chunk.T @ Q.T-block     -> PSUM [128, 512]
  P.T             = exp(scores.T / 8)           -> SBUF bf16 (ScalarE)
  out.T [65, 512] = sum_chunks V'-chunk.T @ P.T -> PSUM accumulate
Rows 0..63 of out.T are the unnormalized numerator, row 64 the softmax
denominator; the host divides and transposes while unsharding.

The scores matmul has contraction dim 64, so pairs of kv-tiles are packed
into the two 64-row halves of the PE array (tile_position row tiling) and
run concurrently; Q.T/K.T are duplicated into partitions 64..127 for this.
"""

import numpy as np
import ml_dtypes

import concourse.bass as bass
import concourse.tile as tile
from concourse import mybir
from concourse.bass_utils import run_bass_kernel_spmd

BF16 = mybir.dt.bfloat16
F32 = mybir.dt.float32

B, S, D, E = 4, 4096, 512, 64
H = S // 2            # q rows per core
E1 = E + 1            # V' width (ones column appended)
NCH = D // 128        # contraction chunks (4)
NKV = S // 128        # kv tiles (32)
NPAIR = NKV // 2      # packed kv tile pairs (16)
QBLK = 512            # sq columns per block
NBLK = H // QBLK      # 4
N_CORES = 8


def _build_bass() -> bass.Bass:
    nc = bass.Bass()
    qT = nc.declare_dram_parameter("qT", [D, H], BF16, isOutput=False)
    kT = nc.declare_dram_parameter("kT", [D, S], BF16, isOutput=False)
    vT = nc.declare_dram_parameter("vT", [D, S], BF16, isOutput=False)
    # weights pre-swizzled on host to [128, chunk*width] (partition-major)
    wq = nc.declare_dram_parameter("wq", [128, NCH * E], BF16, isOutput=False)
    wk = nc.declare_dram_parameter("wk", [128, NCH * E], BF16, isOutput=False)
    wv = nc.declare_dram_parameter("wv", [128, NCH * E1], BF16, isOutput=False)
    bq = nc.declare_dram_parameter("bqb", [E, 512], F32, isOutput=False)
    bvb = nc.declare_dram_parameter("bvb", [128, E1], F32, isOutput=False)
    out = nc.declare_dram_parameter("out", [E1, H], F32, isOutput=True)

    with tile.TileContext(nc) as tc:
        _body(nc, tc, qT, kT, vT, wq, wk, wv, bq, bvb, out)
    _split_multi_waits(nc)
    return nc


_NO_SPLIT_OPCODES = {"Drain", "EventSemaphore", "NoOp", "Call", "ISA",
                     "UnconditionalBranch"}


def _split_multi_waits(nc):
    """walrus (this toolchain) encodes at most ONE sem wait per TPB
    instruction (single NEURON_ISA_TPB_EVENTS slot) and refuses to compile
    instructions carrying more. Tile emits multi-wait sync_info freely, so
    split: keep the first wait on the instruction, hoist the rest onto
    standalone EventSemaphore waits just before it on the same engine."""
    n = 0
    for blk in nc.m.functions[0].blocks:
        new_insts = []
        for inst in blk.instructions:
            si = inst.sync_info
            if (si is not None and si.on_wait and len(si.on_wait) > 1
                    and inst.concise_opcode not in _NO_SPLIT_OPCODES):
                waits = list(si.on_wait)
                for w in waits[:-1]:
                    n += 1
                    es = mybir.InstEventSemaphore(
                        name=f"WSPLIT-{n}", ins=[], outs=[])
                    es.engine = inst.engine
                    es.sync_info = mybir.SyncInfo(on_wait=[w], on_update=[])
                    new_insts.append(es)
                inst.sync_info = mybir.SyncInfo(
                    on_wait=[waits[-1]], on_update=list(si.on_update))
            new_insts.append(inst)
        blk.instructions = new_insts


def _body(nc, tc, qT, kT, vT, wq, wk, wv, bq, bvb, out):
    with (
        tc.tile_pool(name="consts", bufs=1) as cst,
        tc.tile_pool(name="raw", bufs=1) as raw,
        tc.tile_pool(name="proj", bufs=1) as proj,
        tc.tile_pool(name="pt", bufs=6) as ptp,
        tc.tile_pool(name="ob", bufs=2) as obp,
        tc.tile_pool(name="ps", bufs=2, space="PSUM") as ps,
        tc.tile_pool(name="pso", bufs=2, space="PSUM") as pso,
    ):
        wq_sb = cst.tile([128, NCH * E], BF16, tag="wq")
        nc.sync.dma_start(out=wq_sb, in_=wq[:, :])
        wk_sb = cst.tile([128, NCH * E], BF16, tag="wk")
        nc.sync.dma_start(out=wk_sb, in_=wk[:, :])
        wv_sb = cst.tile([128, NCH * E1], BF16, tag="wv")
        nc.sync.dma_start(out=wv_sb, in_=wv[:, :])
        bq_sb = cst.tile([E, 512], F32, tag="bq")
        nc.sync.dma_start(out=bq_sb, in_=bq[:, :])
        bvb_sb = cst.tile([128, E1], F32, tag="bvb")
        nc.sync.dma_start(out=bvb_sb, in_=bvb[:, :])

        # raw transposed inputs, one SBUF tile per 128-row contraction chunk
        qT_sb, kT_sb, vT_sb = [], [], []
        for c in range(NCH):
            t = raw.tile([128, H], BF16, tag=f"qT{c}")
            nc.sync.dma_start(out=t, in_=qT[c * 128:(c + 1) * 128, :])
            qT_sb.append(t)
        for c in range(NCH):
            t = raw.tile([128, S], BF16, tag=f"kT{c}")
            nc.sync.dma_start(out=t, in_=kT[c * 128:(c + 1) * 128, :])
            kT_sb.append(t)
        for c in range(NCH):
            t = raw.tile([128, S], BF16, tag=f"vT{c}")
            nc.sync.dma_start(out=t, in_=vT[c * 128:(c + 1) * 128, :])
            vT_sb.append(t)

        # projected tensors; Q.T/K.T duplicated into partitions 64..127 so
        # the scores matmuls can row-pack both PE array halves
        QT2 = proj.tile([128, H], BF16, tag="QT2")
        KT2 = proj.tile([128, S], BF16, tag="KT2")
        Vp = proj.tile([128, NKV, E1], BF16, tag="Vp")

        # Q.T = Wq.T @ qT (+bq)
        for blk in range(H // 512):
            acc = ps.tile([E, 512], F32, tag="ps_main")
            sl = slice(blk * 512, (blk + 1) * 512)
            for c in range(NCH):
                nc.tensor.matmul(
                    acc[:, :], wq_sb[:, c * E:(c + 1) * E], qT_sb[c][:, sl],
                    start=(c == 0), stop=(c == NCH - 1),
                )
            nc.vector.tensor_add(QT2[0:E, sl], acc[:, :], bq_sb[:, :])
            nc.sync.dma_start(out=QT2[E:2 * E, sl], in_=QT2[0:E, sl])

        # K.T = Wk.T @ kT (bias dropped: constant along kv is softmax-invariant)
        for blk in range(S // 512):
            acc = ps.tile([E, 512], F32, tag="ps_main")
            sl = slice(blk * 512, (blk + 1) * 512)
            for c in range(NCH):
                nc.tensor.matmul(
                    acc[:, :], wk_sb[:, c * E:(c + 1) * E], kT_sb[c][:, sl],
                    start=(c == 0), stop=(c == NCH - 1),
                )
            nc.vector.tensor_copy(KT2[0:E, sl], acc[:, :])
            nc.sync.dma_start(out=KT2[E:2 * E, sl], in_=KT2[0:E, sl])

        # V' tiles [128, 65] = vT-chunk.T @ Wv_aug + bias (col 64 -> 1.0)
        for t in range(NKV):
            acc = ps.tile([128, E1], F32, tag="ps_main")
            for c in range(NCH):
                nc.tensor.matmul(
                    acc[:, :], vT_sb[c][:, t * 128:(t + 1) * 128],
                    wv_sb[:, c * E1:(c + 1) * E1],
                    start=(c == 0), stop=(c == NCH - 1),
                )
            nc.vector.tensor_add(Vp[:, t, :], acc[:, :], bvb_sb[:, :])

        # attention: per sq block, stream kv tile pairs through
        # scores -> exp -> AV accumulate
        for blk in range(NBLK):
            sq = slice(blk * QBLK, (blk + 1) * QBLK)
            acc_o = pso.tile([E1, QBLK], F32, tag="ps_out")
            for p in range(NPAIR):
                sc = ps.tile([128, 2 * QBLK], F32, tag="ps_sc")
                nc.tensor.matmul(
                    sc[:, 0:QBLK],
                    KT2[0:E, (2 * p) * 128:(2 * p + 1) * 128],
                    QT2[0:E, sq],
                    start=True, stop=True, tile_position=(0, 0),
                )
                nc.tensor.matmul(
                    sc[:, QBLK:2 * QBLK],
                    KT2[E:2 * E, (2 * p + 1) * 128:(2 * p + 2) * 128],
                    QT2[E:2 * E, sq],
                    start=True, stop=True, tile_position=(64, 0),
                )
                pt = ptp.tile([128, 2 * QBLK], BF16, tag="pt")
                nc.scalar.activation(
                    pt[:, :], sc[:, :], mybir.ActivationFunctionType.Exp,
                    scale=0.125,
                )
                nc.tensor.matmul(
                    acc_o[:, :], Vp[:, 2 * p, :], pt[:, 0:QBLK],
                    start=(p == 0), stop=False,
                )
                nc.tensor.matmul(
                    acc_o[:, :], Vp[:, 2 * p + 1, :], pt[:, QBLK:2 * QBLK],
                    start=False, stop=(p == NPAIR - 1),
                )
            ob = obp.tile([E1, QBLK], F32, tag="ob")
            nc.vector.tensor_copy(ob[:, :], acc_o[:, :])
            nc.sync.dma_start(out=out[:, sq], in_=ob[:, :])


_CACHED_NC = None


def _get_nc():
    global _CACHED_NC
    if _CACHED_NC is None:
        _CACHED_NC = _build_bass()
    return _CACHED_NC


def _swizzle_w(w: np.ndarray) -> np.ndarray:
    """[512, width] -> [128, NCH*width] with chunk-major free dim."""
    width = w.shape[1]
    return np.ascontiguousarray(
        w.reshape(NCH, 128, width).transpose(1, 0, 2).reshape(128, NCH * width)
    ).astype(ml_dtypes.bfloat16)


def _make_in_maps(q, k, v, Wq, bq, Wk, bk, Wv, bv):
    del bk  # constant along the kv axis -> softmax-invariant, dropped
    bf = ml_dtypes.bfloat16
    wq_s = _swizzle_w(np.asarray(Wq, np.float32))
    wk_s = _swizzle_w(np.asarray(Wk, np.float32))
    wv_aug = np.concatenate(
        [np.asarray(Wv, np.float32), np.zeros((D, 1), np.float32)], axis=1
    )
    wv_s = _swizzle_w(wv_aug)
    bq_a = np.ascontiguousarray(
        np.broadcast_to(np.asarray(bq, np.float32).reshape(E, 1), (E, 512)))
    bvb_row = np.concatenate([np.asarray(bv, np.float32), [1.0]]).astype(np.float32)
    bvb_a = np.ascontiguousarray(np.broadcast_to(bvb_row, (128, E1)))

    in_maps = []
    for core in range(N_CORES):
        b, h = core // 2, core % 2
        qh = np.asarray(q[b, h * H:(h + 1) * H, :], np.float32)
        in_maps.append({
            "qT": np.ascontiguousarray(qh.T).astype(bf),
            "kT": np.ascontiguousarray(np.asarray(k[b], np.float32).T).astype(bf),
            "vT": np.ascontiguousarray(np.asarray(v[b], np.float32).T).astype(bf),
            "wq": wq_s, "wk": wk_s, "wv": wv_s,
            "bqb": bq_a, "bvb": bvb_a,
        })
    return in_maps


def _unshard(results) -> np.ndarray:
    final = np.empty((B, S, E), np.float32)
    for core in range(N_CORES):
        o = np.asarray(results[core]["out"], np.float32)  # [65, H]
        b, h = core // 2, core % 2
        final[b, h * H:(h + 1) * H, :] = (o[:E] / o[E:E + 1]).T
    return final


def kernel(q, k, v, Wq, bq, Wk, bk, Wv, bv, _trace=False):
    nc = _get_nc()
    in_maps = _make_in_maps(q, k, v, Wq, bq, Wk, bk, Wv, bv)
    res = run_bass_kernel_spmd(nc, in_maps, core_ids=list(range(N_CORES)),
                               trace=_trace)
    outp = _unshard(res.results)
    if _trace:
        kernel.last_result = res
    return outp


# revision 9
# speedup vs baseline: 1.0412x; 1.0412x over previous
"""Trainium2 Bass kernel for batched single-head attention with projections.

Reference computation (per batch b):
    Q = q @ Wq + bq ; K = k @ Wk + bk ; V = v @ Wv + bv        (512 -> 64)
    out = softmax(Q K^T / 8) V                                  (S = 4096)

Sharding: 8 cores = 4 batches x 2 query-sequence halves. Each core gets
its q half (transposed, bf16) plus the full k,v for its batch (transposed,
bf16, duplicated across the pair of cores that share the batch).

Device-side layout trick: everything is computed in "transposed space".
  Q.T [64, 2048]  = Wq.T @ qT   (+bq per-partition)
  K.T [64, 4096]  = Wk.T @ kT   (bk dropped: softmax-invariant)
  V'  [4096, 65]  = (vT.T @ Wv_aug) + bias ; col 64 == 1.0 (denominator col)
  scores.T tile   = K.T-chunk.T @ Q.T-block     -> PSUM [128, 512]
  P.T             = exp(scores.T / 8)           -> SBUF bf16 (ScalarE)
  out.T [65, 512] = sum_chunks V'-chunk.T @ P.T -> PSUM accumulate
Rows 0..63 of out.T are the unnormalized numerator, row 64 the softmax
denominator; the host divides and transposes while unsharding.

The scores matmul has contraction dim 64, so pairs of kv-tiles are packed
into the two 64-row halves of the PE array (tile_position row tiling) and
run concurrently; Q.T/K.T are duplicated into partitions 64..127 for this.
"""

import numpy as np
import ml_dtypes

import concourse.bass as bass
import concourse.tile as tile
from concourse import mybir
from concourse.bass_utils import run_bass_kernel_spmd

BF16 = mybir.dt.bfloat16
F32 = mybir.dt.float32

B, S, D, E = 4, 4096, 512, 64
H = S // 2            # q rows per core
E1 = E + 1            # V' width (ones column appended)
NCH = D // 128        # contraction chunks (4)
NKV = S // 128        # kv tiles (32)
NPAIR = NKV // 2      # packed kv tile pairs (16)
QBLK = 512            # sq columns per block
NBLK = H // QBLK      # 4
N_CORES = 8


def _build_bass(split_waits: bool = True) -> bass.Bass:
    nc = bass.Bass()
    qT = nc.declare_dram_parameter("qT", [D, H], BF16, isOutput=False)
    kT = nc.declare_dram_parameter("kT", [D, S], BF16, isOutput=False)
    vT = nc.declare_dram_parameter("vT", [D, S], BF16, isOutput=False)
    # weights pre-swizzled on host to [128, chunk*width] (partition-major)
    wq = nc.declare_dram_parameter("wq", [128, NCH * E], BF16, isOutput=False)
    wk = nc.declare_dram_parameter("wk", [128, NCH * E], BF16, isOutput=False)
    wv = nc.declare_dram_parameter("wv", [128, NCH * E1], BF16, isOutput=False)
    bq = nc.declare_dram_parameter("bqb", [E, 512], F32, isOutput=False)
    bvb = nc.declare_dram_parameter("bvb", [128, E1], F32, isOutput=False)
    out = nc.declare_dram_parameter("out", [E1, H], F32, isOutput=True)

    with tile.TileContext(nc) as tc:
        _body(nc, tc, qT, kT, vT, wq, wk, wv, bq, bvb, out)
    if split_waits:
        _split_multi_waits(nc)
    return nc


_NO_SPLIT_OPCODES = {"Drain", "EventSemaphore", "NoOp", "Call", "ISA",
                     "UnconditionalBranch"}


def _split_multi_waits(nc):
    """walrus (this toolchain) encodes at most ONE sem wait per TPB
    instruction (single NEURON_ISA_TPB_EVENTS slot) and refuses to compile
    instructions carrying more. Tile emits multi-wait sync_info freely, so
    split: keep the first wait on the instruction, hoist the rest onto
    standalone EventSemaphore waits just before it on the same engine."""
    n = 0
    for blk in nc.m.functions[0].blocks:
        new_insts = []
        for inst in blk.instructions:
            si = inst.sync_info
            if (si is not None and si.on_wait and len(si.on_wait) > 1
                    and inst.concise_opcode not in _NO_SPLIT_OPCODES):
                waits = list(si.on_wait)
                for w in waits[:-1]:
                    n += 1
                    es = mybir.InstEventSemaphore(
                        name=f"WSPLIT-{n}", ins=[], outs=[])
                    es.engine = inst.engine
                    es.sync_info = mybir.SyncInfo(on_wait=[w], on_update=[])
                    new_insts.append(es)
                inst.sync_info = mybir.SyncInfo(
                    on_wait=[waits[-1]], on_update=list(si.on_update))
            new_insts.append(inst)
        blk.instructions = new_insts


def _body(nc, tc, qT, kT, vT, wq, wk, wv, bq, bvb, out):
    with (
        tc.tile_pool(name="consts", bufs=1) as cst,
        tc.tile_pool(name="raw", bufs=1) as raw,
        tc.tile_pool(name="proj", bufs=1) as proj,
        tc.tile_pool(name="pt", bufs=8) as ptp,
        tc.tile_pool(name="ob", bufs=2) as obp,
        tc.tile_pool(name="ps", bufs=2, space="PSUM") as ps,
        tc.tile_pool(name="pso", bufs=2, space="PSUM") as pso,
    ):
        wq_sb = cst.tile([128, NCH * E], BF16, tag="wq")
        nc.sync.dma_start(out=wq_sb, in_=wq[:, :])
        wk_sb = cst.tile([128, NCH * E], BF16, tag="wk")
        nc.sync.dma_start(out=wk_sb, in_=wk[:, :])
        wv_sb = cst.tile([128, NCH * E1], BF16, tag="wv")
        nc.sync.dma_start(out=wv_sb, in_=wv[:, :])
        bq_sb = cst.tile([E, 512], F32, tag="bq")
        nc.sync.dma_start(out=bq_sb, in_=bq[:, :])
        bvb_sb = cst.tile([128, E1], F32, tag="bvb")
        nc.sync.dma_start(out=bvb_sb, in_=bvb[:, :])

        # raw transposed inputs, column-sliced [128, 1024] and DMA'd in
        # consumption order so the attention pipeline starts early:
        # qT half 0, kT quarter 0, vT quarter 0, qT half 1, then remaining
        # kT/vT quarters interleaved.
        QW = 1024
        qTs = [[None] * (H // QW) for _ in range(NCH)]
        kTs = [[None] * (S // QW) for _ in range(NCH)]
        vTs = [[None] * (S // QW) for _ in range(NCH)]

        def load(dst, src, grid, c, q):
            t = raw.tile([128, QW], BF16, tag=f"{dst}{c}_{q}")
            nc.sync.dma_start(
                out=t, in_=src[c * 128:(c + 1) * 128, q * QW:(q + 1) * QW])
            grid[c][q] = t

        for c in range(NCH):
            load("q", qT, qTs, c, 0)
        for c in range(NCH):
            load("k", kT, kTs, c, 0)
        for c in range(NCH):
            load("v", vT, vTs, c, 0)
        for c in range(NCH):
            load("q", qT, qTs, c, 1)
        for q in range(1, S // QW):
            for c in range(NCH):
                load("k", kT, kTs, c, q)
            for c in range(NCH):
                load("v", vT, vTs, c, q)

        # projected tensors; Q.T/K.T duplicated into partitions 64..127 so
        # the scores matmuls can row-pack both PE array halves
        QT2 = proj.tile([128, H], BF16, tag="QT2")
        KT2 = proj.tile([128, S], BF16, tag="KT2")
        Vp = proj.tile([128, NKV, E1], BF16, tag="Vp")

        # projections, emitted in dependency-arrival order; the dup DMAs
        # (partitions 64..127 copies) ride the ACT DGE ring so they never
        # queue behind the big input loads on the sync ring
        def q_proj(blk):
            acc = ps.tile([E, 512], F32, tag="ps_main")
            sl = slice(blk * 512, (blk + 1) * 512)
            for c in range(NCH):
                nc.tensor.matmul(
                    acc[:, :], wq_sb[:, c * E:(c + 1) * E],
                    qTs[c][blk // 2][:, (blk % 2) * 512:(blk % 2 + 1) * 512],
                    start=(c == 0), stop=(c == NCH - 1),
                )
            nc.vector.tensor_add(QT2[0:E, sl], acc[:, :], bq_sb[:, :])
            nc.scalar.dma_start(out=QT2[E:2 * E, sl], in_=QT2[0:E, sl])

        def k_proj(blk):
            acc = ps.tile([E, 512], F32, tag="ps_main")
            sl = slice(blk * 512, (blk + 1) * 512)
            for c in range(NCH):
                nc.tensor.matmul(
                    acc[:, :], wk_sb[:, c * E:(c + 1) * E],
                    kTs[c][blk // 2][:, (blk % 2) * 512:(blk % 2 + 1) * 512],
                    start=(c == 0), stop=(c == NCH - 1),
                )
            nc.vector.tensor_copy(KT2[0:E, sl], acc[:, :])
            nc.scalar.dma_start(out=KT2[E:2 * E, sl], in_=KT2[0:E, sl])

        def v_proj(t):
            acc = ps.tile([128, E1], F32, tag="ps_main")
            for c in range(NCH):
                nc.tensor.matmul(
                    acc[:, :],
                    vTs[c][t // 8][:, (t % 8) * 128:(t % 8 + 1) * 128],
                    wv_sb[:, c * E1:(c + 1) * E1],
                    start=(c == 0), stop=(c == NCH - 1),
                )
            nc.vector.tensor_add(Vp[:, t, :], acc[:, :], bvb_sb[:, :])

        q_proj(0); q_proj(1)
        k_proj(0); k_proj(1)
        for t in range(8):
            v_proj(t)
        q_proj(2); q_proj(3)
        for q in range(1, 4):
            k_proj(2 * q); k_proj(2 * q + 1)
            for t in range(8 * q, 8 * q + 8):
                v_proj(t)

        # attention: per sq block, stream kv tile pairs through
        # scores -> exp -> AV accumulate
        for blk in range(NBLK):
            sq = slice(blk * QBLK, (blk + 1) * QBLK)
            acc_o = pso.tile([E1, QBLK], F32, tag="ps_out")
            for p in range(NPAIR):
                sc = ps.tile([128, 2 * QBLK], F32, tag="ps_sc")
                nc.tensor.matmul(
                    sc[:, 0:QBLK],
                    KT2[0:E, (2 * p) * 128:(2 * p + 1) * 128],
                    QT2[0:E, sq],
                    start=True, stop=True, tile_position=(0, 0),
                )
                nc.tensor.matmul(
                    sc[:, QBLK:2 * QBLK],
                    KT2[E:2 * E, (2 * p + 1) * 128:(2 * p + 2) * 128],
                    QT2[E:2 * E, sq],
                    start=True, stop=True, tile_position=(64, 0),
                )
                pt = ptp.tile([128, 2 * QBLK], BF16, tag="pt")
                nc.scalar.activation(
                    pt[:, :], sc[:, :], mybir.ActivationFunctionType.Exp,
                    scale=0.125,
                )
                nc.tensor.matmul(
                    acc_o[:, :], Vp[:, 2 * p, :], pt[:, 0:QBLK],
                    start=(p == 0), stop=False,
                )
                nc.tensor.matmul(
                    acc_o[:, :], Vp[:, 2 * p + 1, :], pt[:, QBLK:2 * QBLK],
                    start=False, stop=(p == NPAIR - 1),
                )
            ob = obp.tile([E1, QBLK], F32, tag="ob")
            nc.vector.tensor_copy(ob[:, :], acc_o[:, :])
            nc.scalar.dma_start(out=out[:, sq], in_=ob[:, :])


_CACHED_NC = None


def _get_nc():
    global _CACHED_NC
    if _CACHED_NC is None:
        _CACHED_NC = _build_bass()
    return _CACHED_NC


def _swizzle_w(w: np.ndarray) -> np.ndarray:
    """[512, width] -> [128, NCH*width] with chunk-major free dim."""
    width = w.shape[1]
    return np.ascontiguousarray(
        w.reshape(NCH, 128, width).transpose(1, 0, 2).reshape(128, NCH * width)
    ).astype(ml_dtypes.bfloat16)


def _make_in_maps(q, k, v, Wq, bq, Wk, bk, Wv, bv):
    del bk  # constant along the kv axis -> softmax-invariant, dropped
    bf = ml_dtypes.bfloat16
    wq_s = _swizzle_w(np.asarray(Wq, np.float32))
    wk_s = _swizzle_w(np.asarray(Wk, np.float32))
    wv_aug = np.concatenate(
        [np.asarray(Wv, np.float32), np.zeros((D, 1), np.float32)], axis=1
    )
    wv_s = _swizzle_w(wv_aug)
    bq_a = np.ascontiguousarray(
        np.broadcast_to(np.asarray(bq, np.float32).reshape(E, 1), (E, 512)))
    bvb_row = np.concatenate([np.asarray(bv, np.float32), [1.0]]).astype(np.float32)
    bvb_a = np.ascontiguousarray(np.broadcast_to(bvb_row, (128, E1)))

    in_maps = []
    for core in range(N_CORES):
        b, h = core // 2, core % 2
        qh = np.asarray(q[b, h * H:(h + 1) * H, :], np.float32)
        in_maps.append({
            "qT": np.ascontiguousarray(qh.T).astype(bf),
            "kT": np.ascontiguousarray(np.asarray(k[b], np.float32).T).astype(bf),
            "vT": np.ascontiguousarray(np.asarray(v[b], np.float32).T).astype(bf),
            "wq": wq_s, "wk": wk_s, "wv": wv_s,
            "bqb": bq_a, "bvb": bvb_a,
        })
    return in_maps


def _unshard(results) -> np.ndarray:
    final = np.empty((B, S, E), np.float32)
    for core in range(N_CORES):
        o = np.asarray(results[core]["out"], np.float32)  # [65, H]
        b, h = core // 2, core % 2
        final[b, h * H:(h + 1) * H, :] = (o[:E] / o[E:E + 1]).T
    return final


def kernel(q, k, v, Wq, bq, Wk, bk, Wv, bv, _trace=False):
    nc = _get_nc()
    in_maps = _make_in_maps(q, k, v, Wq, bq, Wk, bk, Wv, bv)
    res = run_bass_kernel_spmd(nc, in_maps, core_ids=list(range(N_CORES)),
                               trace=_trace)
    outp = _unshard(res.results)
    if _trace:
        kernel.last_result = res
    return outp
